# revision 1
# baseline (speedup 1.0000x reference)
"""Trainium2 Bass kernel for nn_BilateralSolverLocal.

loss = H*W*LAM * mean(w_ij * d^2) + mean((output-target)^2)
  where d[c,i,j] = output[i+10, j+10] - output[i+ci, j+cj] + bias[c]
  (c enumerates the K*K-1 = 440 non-center taps (ci,cj) of a 21x21 window,
   row-major with the center (10,10) removed; bias is zeros per the spec).

Sharding: 8 cores = 4 row-blocks (123 output rows) x 2 column-halves
(246 output cols).  Every core runs the IDENTICAL program over all 440
channels of its spatial patch; per-core inputs differ only in data, so a
single SPMD Bass program works.  Final loss = host sum of 8 partial sums.

Per-core pipeline (all heavy arrays stream through bf16; the host
pre-transposes w to [row, channel, col], pre-casts it to bf16, and
pre-builds the 21 partition-shifted image slab copies in two parity
variants so every window start stays 4B-aligned and the DVE tensor ops
run in their 2x bf16 mode; HWDGE-only DMAs with per-partition-contiguous
runs, since SWDGE descriptor emission and sub-KB runs are descriptor-
rate-bound):
  - DVE: d = center - window  (one op covers a whole stride-2 cj run via a
    broadcast AP and an overlapping-window AP)
  - ACT: d2 = Square(d)
  - DVE: u = d2 * w
  - PE : ones[123,1]^T @ u accumulated into one PSUM [1,492] tile
  - fidelity partial on each core's 64-row stripe (f32)
"""

import os
import sys

import numpy as np

H = 512
W = 512
K = 21
C0 = (K - 1) // 2          # 10
NCH = K * K - 1            # 440
LAM = 128.0
HO = H - K + 1             # 492
WO = W - K + 1             # 492

P = 123                    # output rows per core block (4 * 123 = 492)
CW = 246                   # output cols per core block (2 * 246 = 492)
PATCH_R = P + K - 1        # 143 image rows per core patch
PATCH_C = CW + K - 1       # 266 image cols per core patch
BW = 264                   # odd-parity mega-slab chunk width (even, >= 18+246)
FR = H // 8                # 64 fidelity rows per core
N_CORES = 8

_CACHE: dict = {}
LAST_EXEC_TIME_NS = None
LAST_RESULTS = None


def _ensure_paths():
    for p in ("/opt/trn_rl_repo", "/root/.axon_site/_ro/trn_rl_repo"):
        if os.path.isdir(p) and p not in sys.path:
            sys.path.append(p)


def _runs_for(cjs):
    """Split slot list into runs where cj steps by 2 and slot steps by 2."""
    runs = []
    used = [False] * len(cjs)
    for s0 in range(len(cjs)):
        if used[s0]:
            continue
        cj0 = cjs[s0]
        chain = [(s0, cj0)]
        used[s0] = True
        s, cj = s0, cj0
        while True:
            # next element with cj+2 at slot s+2
            nxt = None
            for s2 in range(s + 1, len(cjs)):
                if not used[s2] and cjs[s2] == cj + 2 and s2 == s + 2:
                    nxt = s2
                    break
            if nxt is None:
                break
            used[nxt] = True
            chain.append((nxt, cj + 2))
            s, cj = nxt, cj + 2
        runs.append((chain[0][1], chain[0][0], len(chain)))
    return runs


def _build_program(repeat=1, stages=5, only=None, dma_split=3, dma_alt=True,
                   w_bufs=3, work_bufs=3, dma_flat=False, gp_sub=0, pe_mul=False):
    """stages: 0 DMAs only, 1 +casts, 2 +sub, 3 +square, 4 +mul, 5 full.
    only: None | 'dma' | 'dve' | 'act' | 'pe' — engine-isolated timing builds
    (ops run on garbage SBUF data; outputs are meaningless)."""
    _ensure_paths()
    import concourse.bass as bass
    import concourse.bacc as bacc
    import concourse.mybir as mybir
    from concourse.tile import TileContext

    f32 = mybir.dt.float32
    bf16 = mybir.dt.bfloat16
    AX = mybir.AxisListType
    OP = mybir.AluOpType

    nc = bacc.Bacc()
    # w is host-pre-transposed to [row, channel, col] AND pre-cast to bf16:
    # halves the HBM read and lets each per-ci DMA read one contiguous
    # nch*246*2B run per partition (descriptor-rate friendly).  The slab
    # tensors (21 partition-shifted copies of the image patch, two parity
    # copies for 4B alignment of odd-cj windows) are also host-built bf16.
    w_in = nc.dram_tensor("w", [P, NCH, CW], bf16, kind="ExternalInput")
    id_in = nc.dram_tensor("ident", [128, 128], bf16, kind="ExternalInput")
    slabA_in = nc.dram_tensor("slabA", [P, K * PATCH_C], bf16, kind="ExternalInput")
    slabB_in = nc.dram_tensor("slabB", [P, K * BW], bf16, kind="ExternalInput")
    fo_in = nc.dram_tensor("fo", [FR, W], f32, kind="ExternalInput")
    ft_in = nc.dram_tensor("ft", [FR, W], f32, kind="ExternalInput")
    out_d = nc.dram_tensor("partials", [1, 2], f32, kind="ExternalOutput")
    nc.default_dma_engine = nc.sync

    # channel bookkeeping
    groups = []
    for ci in range(K):
        if ci < C0:
            c_first, cjs = ci * K, list(range(K))
        elif ci == C0:
            c_first, cjs = C0 * K, [cj for cj in range(K) if cj != C0]
        else:
            c_first, cjs = ci * K - 1, list(range(K))
        groups.append((ci, c_first, cjs, _runs_for(cjs)))

    n_mm_total = NCH  # one matmul (N=246) per channel
    AW = K * PATCH_C  # mega slab A free width (per partition)
    BWW = K * BW      # mega slab B free width

    full = only is None
    use_dma = (full and stages >= 0) or only == "dma"
    # engine-isolated builds read tiles nobody writes; a 1-column gpsimd
    # memset "allocates" them for Tile without polluting the timed engine
    use_sub = (full and stages >= 2) or only == "dve"
    use_sq = (full and stages >= 3) or only == "act"
    use_mul = (full and stages >= 4) or only == "dve"
    use_pe = (full and stages >= 5) or only == "pe"
    use_fid = full and stages >= 5

    with TileContext(nc) as tc:
        with (
            tc.tile_pool(name="singles", bufs=1) as singles,
            tc.tile_pool(name="wpool", bufs=w_bufs) as wpool,
            tc.tile_pool(name="work", bufs=work_bufs) as work,
            tc.tile_pool(name="fid", bufs=1) as fidp,
            tc.tile_pool(name="psum", bufs=1, space="PSUM") as psump,
        ):
            ones_bf = singles.tile([128, 1], bf16, tag="ones_bf")
            nc.vector.memset(ones_bf, 1.0)
            ident = singles.tile([128, 128], bf16, tag="ident")
            if pe_mul:
                nc.sync.dma_start(out=ident[:, :], in_=id_in[:, :])
            ones_f32 = singles.tile([128, 1], f32, tag="ones_f32")
            nc.vector.memset(ones_f32, 1.0)

            # ---- fidelity on this core's 64-row stripe (f32, cheap) ----
            psum_f = psump.tile([1, 256], f32, tag="psum_f")
            if use_fid:
                fo_t = fidp.tile([128, 256], f32, tag="fo")
                ft_t = fidp.tile([128, 256], f32, tag="ft")
                src_fo = bass.AP(
                    tensor=fo_in, offset=0, ap=[[W, FR], [256, 2], [1, 256]]
                )
                src_ft = bass.AP(
                    tensor=ft_in, offset=0, ap=[[W, FR], [256, 2], [1, 256]]
                )
                nc.sync.dma_start(out=fo_t[:, :], in_=src_fo)
                nc.sync.dma_start(out=ft_t[:, :], in_=src_ft)
                fd = fidp.tile([128, 256], f32, tag="fd")
                nc.vector.tensor_sub(out=fd[:, :], in0=fo_t[:, :], in1=ft_t[:, :])
                fd2 = fidp.tile([128, 256], f32, tag="fd2")
                nc.scalar.square(out=fd2[:, :], in_=fd[:, :])
                nc.tensor.matmul(
                    psum_f[0:1, :], ones_f32[0:128, 0:1], fd2[:, :],
                    start=True, stop=True,
                )

            # ---- mega slabs (host-built bf16, plain contiguous loads) ----
            slabA = singles.tile([P, AW], bf16, tag="slabA")
            slabB = singles.tile([P, BWW], bf16, tag="slabB")
            if use_dma:
                nc.sync.dma_start(out=slabA[:, :], in_=slabA_in[:, :])
                nc.scalar.dma_start(out=slabB[:, :], in_=slabB_in[:, :])
            elif use_sub:
                nc.gpsimd.memset(slabA[:, 0:1], 0.0)
                nc.gpsimd.memset(slabB[:, 0:1], 0.0)

            # center view: A chunk g=10, col offset 10
            cen = slabA[:, C0 * PATCH_C + C0: C0 * PATCH_C + C0 + 1]

            if pe_mul:
                psum_s = psump.tile([128, 128], f32, tag="psum_s")
            else:
                psum_s = psump.tile([1, 2 * CW], f32, tag="psum_s")
            n_mm_total = (2 * NCH) if pe_mul else NCH

            mm_idx = 0
            n_mm_total *= repeat
            for _rep in range(repeat):
              for gi, (ci, c_first, cjs, runs) in enumerate(groups):
                nch = len(cjs)
                wt = wpool.tile([P, nch * CW], bf16, tag="wt")
                if use_dma:
                    step = (nch + dma_split - 1) // dma_split
                    splits = list(range(0, nch, step)) + [nch]
                    for si, (a, b) in enumerate(zip(splits[:-1], splits[1:])):
                        if a >= b:
                            continue
                        sw = bass.AP(
                            tensor=w_in, offset=(c_first + a) * CW,
                            ap=[[NCH * CW, P], [CW, b - a], [1, CW]],
                        )
                        eng = nc.scalar if (dma_alt and si % 2) else nc.sync
                        eng.dma_start(out=wt[:, a * CW: b * CW], in_=sw)
                elif use_mul:
                    nc.gpsimd.memset(wt[:, 0:1], 0.0)

                for (cj0, slot0, n) in runs:
                    if not (use_sub or use_sq or use_mul or use_pe):
                        continue
                    # window source AP (overlapping, stride-2-element outer dim)
                    if cj0 % 2 == 0:
                        base = slabA[:, ci * PATCH_C + cj0: ci * PATCH_C + cj0 + 1]
                    else:
                        base = slabB[:, ci * BW + (cj0 - 1): ci * BW + cj0]
                    win = bass.AP(
                        tensor=base.tensor, offset=base.offset,
                        ap=[list(base.ap[0]), [2, n], [1, CW]],
                    )
                    cview = bass.AP(
                        tensor=cen.tensor, offset=cen.offset,
                        ap=[list(cen.ap[0]), [0, n], [1, CW]],
                    )
                    dt = work.tile([P, n * CW], bf16, tag="dt")
                    dt3 = bass.AP(
                        tensor=dt.tensor, offset=dt.offset,
                        ap=[list(dt.ap[0]), [CW, n], [1, CW]],
                    )
                    if use_sub:
                        veng = (
                            nc.gpsimd
                            if (gp_sub and gi % gp_sub == gp_sub - 1)
                            else nc.vector
                        )
                        veng.tensor_sub(out=dt3, in0=cview, in1=win)
                    elif use_sq:
                        nc.gpsimd.memset(dt[:, 0:1], 0.0)
                    d2 = work.tile([P, n * CW], bf16, tag="d2")
                    if use_sq:
                        nc.scalar.square(out=d2[:, :], in_=dt[:, :])
                    elif use_mul:
                        nc.gpsimd.memset(d2[:, 0:1], 0.0)
                    # w view: channel slots slot0, slot0+2, ... (stride 2*CW)
                    wbase = wt[:, slot0 * CW: slot0 * CW + 1]
                    wview = bass.AP(
                        tensor=wbase.tensor, offset=wbase.offset,
                        ap=[list(wbase.ap[0]), [2 * CW, n], [1, CW]],
                    )
                    ut = work.tile([P, n * CW], bf16, tag="ut")
                    ut3 = bass.AP(
                        tensor=ut.tensor, offset=ut.offset,
                        ap=[list(ut.ap[0]), [CW, n], [1, CW]],
                    )
                    d23 = bass.AP(
                        tensor=d2.tensor, offset=d2.offset,
                        ap=[list(d2.ap[0]), [CW, n], [1, CW]],
                    )
                    if pe_mul:
                        # PE computes w^T @ d2 per 128/118-col channel chunk,
                        # all accumulated into ONE [128,128] psum; only the
                        # diagonal (= per-column sums of w*d2) is meaningful
                        if use_pe:
                            for k in range(n):
                                # uniform 123-col chunks: every matmul covers
                                # the same [0:123]^2 psum region (clean
                                # start/stop) and never triggers FWL (which
                                # only engages at exactly 128 weight columns)
                                chunks = ((0, 123), (123, 123))
                                for off, c in chunks:
                                    wl = wt[:, (slot0 + 2 * k) * CW + off:
                                            (slot0 + 2 * k) * CW + off + c]
                                    rl = d2[:, k * CW + off: k * CW + off + c]
                                    nc.tensor.matmul(
                                        psum_s[0:c, 0:c], wl, rl,
                                        start=(mm_idx == 0),
                                        stop=(mm_idx + 1 == n_mm_total),
                                    )
                                    mm_idx += 1
                    else:
                        if use_mul:
                            nc.vector.tensor_mul(out=ut3, in0=d23, in1=wview)
                        elif use_pe:
                            nc.gpsimd.memset(ut[:, 0:1], 0.0)
                        k = 0
                        while use_pe and k < n:
                            kk = min(2, n - k)
                            nc.tensor.matmul(
                                psum_s[0:1, 0: kk * CW],
                                ones_bf[0:P, 0:1],
                                ut[:, k * CW:(k + kk) * CW],
                                start=(mm_idx == 0),
                                stop=(mm_idx + kk == n_mm_total),
                            )
                            mm_idx += kk
                            k += kk

            # ---- final reduction + store ----
            if use_fid and use_pe:
                res = singles.tile([1, 2], f32, tag="res")
                if pe_mul:
                    # diag(psum_s) summed per partition, then across partitions
                    diag = singles.tile([128, 1], f32, tag="diag")
                    scr = singles.tile([128, 128], f32, tag="scr")
                    nc.vector.tensor_tensor_reduce(
                        out=scr[0:P, 0:P],
                        in0=psum_s[0:P, 0:P], in1=ident[0:P, 0:P],
                        scale=1.0, scalar=0.0,
                        op0=OP.mult, op1=OP.add,
                        accum_out=diag[0:P, :],
                    )
                    psum_d = psump.tile([1, 1], f32, tag="psum_d")
                    nc.tensor.matmul(
                        psum_d[0:1, 0:1], ones_f32[0:P, 0:1], diag[0:P, 0:1],
                        start=True, stop=True,
                    )
                    nc.vector.tensor_copy(
                        out=res[0:1, 0:1], in_=psum_d[0:1, 0:1]
                    )
                else:
                    nc.vector.reduce_sum(
                        out=res[0:1, 0:1], in_=psum_s[0:1, :], axis=AX.X
                    )
                nc.vector.reduce_sum(
                    out=res[0:1, 1:2], in_=psum_f[0:1, :], axis=AX.X
                )
                nc.sync.dma_start(out=out_d[0:1, :], in_=res[0:1, :])

    if not nc.is_finalized():
        nc.finalize()
    return nc


def _numpy_fallback(output, target, w_ij, bias):
    """Exact reference in numpy (streamed per channel); only used if bias!=0."""
    output = np.asarray(output, np.float32)
    target = np.asarray(target, np.float32)
    w_ij = np.asarray(w_ij, np.float32)
    bias = np.asarray(bias, np.float32)
    acc = np.float64(0.0)
    c = 0
    for t in range(K * K):
        ci, cj = t // K, t % K
        if ci == C0 and cj == C0:
            continue
        d = (
            output[C0: C0 + HO, C0: C0 + WO]
            - output[ci: ci + HO, cj: cj + WO]
            + bias[c]
        )
        acc += np.sum((w_ij[c] * d * d).astype(np.float64))
        c += 1
    smooth = H * W * LAM * acc / (NCH * HO * WO)
    fid = np.mean((output - target) ** 2, dtype=np.float64)
    return np.float32(smooth + fid)


def kernel(output, target, w_ij, bias):
    global LAST_EXEC_TIME_NS, LAST_RESULTS
    output = np.ascontiguousarray(np.asarray(output, dtype=np.float32))
    target = np.ascontiguousarray(np.asarray(target, dtype=np.float32))
    w_ij = np.asarray(w_ij, dtype=np.float32)
    bias = np.asarray(bias, dtype=np.float32)

    if np.any(bias != 0):
        return _numpy_fallback(output, target, w_ij, bias)

    _ensure_paths()
    from concourse.bass_utils import run_bass_kernel_spmd

    if "nc" not in _CACHE:
        _CACHE["nc"] = _build_program()
    nc = _CACHE["nc"]

    in_maps = _make_in_maps(output, target, w_ij)

    trace_dir = os.environ.get("KERNEL_TRACE_DIR")
    kwargs = {}
    if trace_dir:
        kwargs = dict(trace=True, tmpdir=trace_dir)
    else:
        # the axon client here lacks the NTFF hook; a stray BASS_TRACE=1
        # would send run_bass_kernel_spmd down an import that fails
        os.environ.setdefault("BASS_NEVER_TRACE", "1")
    res = run_bass_kernel_spmd(nc, in_maps, list(range(N_CORES)), **kwargs)
    LAST_EXEC_TIME_NS = res.exec_time_ns
    LAST_RESULTS = res

    smooth_sum = np.float64(0.0)
    fid_sum = np.float64(0.0)
    for m in range(N_CORES):
        p = np.asarray(res.results[m]["partials"], np.float64)
        smooth_sum += p[0, 0]
        fid_sum += p[0, 1]
    loss = H * W * LAM * smooth_sum / (NCH * HO * WO) + fid_sum / (H * W)
    return np.float32(loss)


def _make_in_maps(output, target, w_ij):
    import ml_dtypes

    bf16 = ml_dtypes.bfloat16
    in_maps = []
    for m in range(N_CORES):
        i0 = P * (m // 2)
        j0 = CW * (m % 2)
        patch = output[i0: i0 + PATCH_R, j0: j0 + PATCH_C].astype(bf16)
        s0, s1 = patch.strides
        slabA = np.lib.stride_tricks.as_strided(
            patch, shape=(P, K, PATCH_C), strides=(s0, s0, s1)
        ).reshape(P, K * PATCH_C)
        slabB = np.lib.stride_tricks.as_strided(
            patch[:, 1:], shape=(P, K, BW), strides=(s0, s0, s1)
        ).reshape(P, K * BW)
        in_maps.append(
            {
                "w": np.ascontiguousarray(
                    w_ij[:, i0: i0 + P, j0: j0 + CW].transpose(1, 0, 2)
                ).astype(bf16),
                "ident": np.eye(128, dtype=bf16),
                "slabA": np.ascontiguousarray(slabA),
                "slabB": np.ascontiguousarray(slabB),
                "fo": np.ascontiguousarray(output[FR * m: FR * (m + 1)]),
                "ft": np.ascontiguousarray(target[FR * m: FR * (m + 1)]),
            }
        )
    return in_maps


class _Runner:
    """Cached shard_map executor: device-resident inputs, repeat dispatch.

    chain=N executes the NEFF N times inside one dispatch, threading each
    call's outputs into the next call's output-donation slots (a real data
    dependency, so the executions serialize on-device).  Used for timing.
    """

    def __init__(self, nc, in_maps, chain=1):
        _ensure_paths()
        import jax
        import numpy as _np
        from jax.experimental.shard_map import shard_map
        from jax.sharding import Mesh, PartitionSpec, NamedSharding
        import concourse.mybir as mybir
        from concourse import bass2jax

        bass2jax.install_neuronx_cc_hook()
        self.jax = jax
        n_cores = len(in_maps)
        partition_name = (
            nc.partition_id_tensor.name if nc.partition_id_tensor else None
        )
        in_names, out_names, out_avals, zero_outs = [], [], [], []
        for alloc in nc.m.functions[0].allocations:
            if not isinstance(alloc, mybir.MemoryLocationSet):
                continue
            name = alloc.memorylocations[0].name
            if alloc.kind == "ExternalInput":
                if name != partition_name:
                    in_names.append(name)
            elif alloc.kind == "ExternalOutput":
                out_names.append(name)
                shape = tuple(alloc.tensor_shape)
                dtype = mybir.dt.np(alloc.dtype)
                out_avals.append(jax.core.ShapedArray(shape, dtype))
                zero_outs.append(_np.zeros(shape, dtype))
        n_params = len(in_names)
        self.out_names = out_names
        self.out_avals = out_avals
        all_in_names = list(in_names) + out_names
        if partition_name is not None:
            all_in_names.append(partition_name)

        def _body(*args):
            params = list(args[:n_params])
            outs = list(args[n_params:])
            for _ in range(chain):
                operands = params + outs
                if partition_name is not None:
                    operands.append(bass2jax.partition_id_tensor())
                outs = list(
                    bass2jax._bass_exec_p.bind(
                        *operands,
                        out_avals=tuple(out_avals),
                        in_names=tuple(all_in_names),
                        out_names=tuple(out_names),
                        lowering_input_output_aliases=(),
                        sim_require_finite=True,
                        sim_require_nnan=True,
                        nc=nc,
                    )
                )
            return tuple(outs)

        devices = jax.devices()[:n_cores]
        mesh = Mesh(_np.asarray(devices), ("core",))
        n_outs = len(out_names)
        in_specs = (PartitionSpec("core"),) * (n_params + n_outs)
        out_specs = (PartitionSpec("core"),) * n_outs
        self.fn = jax.jit(
            shard_map(
                _body, mesh=mesh, in_specs=in_specs, out_specs=out_specs,
                check_rep=False,
            ),
            keep_unused=True,
        )
        sharding = NamedSharding(mesh, PartitionSpec("core"))
        self.dev_in = [
            jax.device_put(
                _np.concatenate([in_maps[c][nm] for c in range(n_cores)], axis=0),
                sharding,
            )
            for nm in in_names
        ]
        self.zeros = [
            jax.device_put(
                _np.zeros((n_cores * z.shape[0], *z.shape[1:]), z.dtype), sharding
            )
            for z in zero_outs
        ]
        self.n_cores = n_cores

    def run(self):
        out = self.fn(*self.dev_in, *self.zeros)
        return out

    def results_np(self, out):
        import numpy as _np
        return [
            {
                nm: _np.asarray(out[i]).reshape(
                    self.n_cores, *self.out_avals[i].shape
                )[c]
                for i, nm in enumerate(self.out_names)
            }
            for c in range(self.n_cores)
        ]

    def time_min(self, iters=10):
        import time as _time
        best = float("inf")
        for _ in range(iters):
            t0 = _time.perf_counter()
            out = self.run()
            self.jax.block_until_ready(out)
            best = min(best, _time.perf_counter() - t0)
        return best


def measure_hw_time_ns(np_inputs, repeat=8, rounds=18):
    """Per-invocation device time via interleaved (T(R) - T(1)) / (R-1) rounds.

    The dispatch floor through the axon tunnel is ~60-80 ms with ms-scale
    drift, so single-shot wall times are useless; the repeat-R program runs
    the whole kernel body R times on-device and the slope isolates it.
    Rounds interleave R=1 / R=R / R=1 to cancel drift; the median is
    reported (noise is roughly +-100 us even so).
    """
    import time as _time
    import jax

    in_maps = _make_in_maps(
        np.asarray(np_inputs["output"], np.float32),
        np.asarray(np_inputs["target"], np.float32),
        np.asarray(np_inputs["w_ij"], np.float32),
    )
    r1 = _Runner(_build_program(repeat=1), in_maps)
    rR = _Runner(_build_program(repeat=repeat), in_maps)
    r1.run()
    rR.run()

    def timed(r):
        t0 = _time.perf_counter()
        jax.block_until_ready(r.run())
        return _time.perf_counter() - t0

    slopes = []
    t1s = []
    tRs = []
    for _ in range(rounds):
        a = timed(r1)
        b = timed(rR)
        a2 = timed(r1)
        t1s.append(min(a, a2))
        tRs.append(b)
        slopes.append((b - (a + a2) / 2) / (repeat - 1))
    med = float(np.median(slopes))
    return med * 1e9, min(t1s) * 1e9, min(tRs) * 1e9

